# revision 10
# baseline (speedup 1.0000x reference)
"""Trainium2 Bass kernel for fused MoE TKG decode (nn_MoEFusedTKG).

Computation (per reference):
  x = rmsnorm(hidden_states) ; logits = x @ router_w ; probs = softmax
  aff = top4-masked probs (not renormalized)
  routed = sum_e aff[:,e] * (silu/GLU MLP_e(x))    (all 16 experts dense)
  shared = SwiGLU shared-expert MLP(x)
  out = routed + shared

Sharding (8 cores):
  - expert-parallel: 2 experts per core (gate_up/down sliced on E)
  - shared expert tensor-parallel on the intermediate dim (512 per core)
  - router/LN replicated; router columns PERMUTED per core so that each
    core's local experts are always affinity columns 0 and 1 (keeps the
    program identical across cores - pure SPMD, no partition_id)
  - final combine: on-device AllReduce(add) over all 8 cores

Layout strategy (decode: only T=32 tokens):
  - activations live transposed ([feature-on-partition, token-on-free])
    as matmul lhsT (stationary); weights stream as rhs in 1 MiB
    contiguous slabs straight from HBM -> kernel is HBM-BW bound.
  - matmuls tagged float32r (single-pass fp32, full PE rate); PSUM
    accumulation stays fp32.
"""

import numpy as np

import concourse.bass as bass
from concourse import bacc
import concourse.mybir as mybir
import concourse.tile as tile
from concourse.bass_utils import run_bass_kernel_spmd
from concourse.masks import make_identity

F32 = mybir.dt.float32
F32R = mybir.dt.float32r
AF = mybir.ActivationFunctionType
ALU = mybir.AluOpType
AX = mybir.AxisListType

B, S, H = 32, 1, 2048
E, I, IS = 16, 4096, 4096
T = B * S                      # 32 tokens
NCORES = 8
EL = E // NCORES               # 2 local experts / core
ISL = IS // NCORES             # 512 shared-intermediate / core
TOPK = 4
EPS = 1e-6

HC = H // 128                  # 16 h-chunks
IC = I // 128                  # 32 i-chunks
GU_COLS = 2 * I                # 8192
USE_F32R = True                # matmul fast mode knob


WDT = F32R if USE_F32R else F32   # dtype tag for matmul operands


def _w(ap):
    # reinterpret a DRAM fp32 AP as the matmul dtype (same bits)
    return ap.bitcast(WDT) if USE_F32R else ap


def build_nc():
    nc = bacc.Bacc(None, num_devices=NCORES)

    hs_d = nc.dram_tensor("hs", [T, H], F32, kind="ExternalInput")
    lnw_d = nc.dram_tensor("lnw", [H], F32, kind="ExternalInput")
    rw_d = nc.dram_tensor("rw", [H, E], F32, kind="ExternalInput")
    gup_d = nc.dram_tensor("gup", [EL, H, GU_COLS], F32, kind="ExternalInput")
    dwn_d = nc.dram_tensor("dwn", [EL, I, H], F32, kind="ExternalInput")
    sgw_d = nc.dram_tensor("sgw", [H, ISL], F32, kind="ExternalInput")
    suw_d = nc.dram_tensor("suw", [H, ISL], F32, kind="ExternalInput")
    sdw_d = nc.dram_tensor("sdw", [ISL, H], F32, kind="ExternalInput")
    out_d = nc.dram_tensor("out", [T, H], F32, kind="ExternalOutput")

    cc_in = nc.dram_tensor("cc_in", [T, H], F32)
    cc_out = nc.dram_tensor("cc_out", [T, H], F32, addr_space="Shared")

    from contextlib import ExitStack

    with tile.TileContext(nc) as tc:
        with ExitStack() as ctx:
            singles = ctx.enter_context(tc.tile_pool(name="singles", bufs=1))
            wslab = ctx.enter_context(tc.tile_pool(name="wslab", bufs=4))
            sbig = ctx.enter_context(tc.tile_pool(name="sbig", bufs=2))
            ps_mm = ctx.enter_context(tc.tile_pool(name="ps_mm", bufs=4, space="PSUM"))
            ps_tr = ctx.enter_context(tc.tile_pool(name="ps_tr", bufs=2, space="PSUM"))
            ps_sm = ctx.enter_context(tc.tile_pool(name="ps_sm", bufs=1, space="PSUM"))
            # ---------------- prologue: inputs ----------------
            hs_t = singles.tile([T, H], F32)
            nc.sync.dma_start(out=hs_t[:], in_=hs_d[:, :])
            lnw_t = singles.tile([128, HC], F32)  # lnw_t[p, h] = lnw[h*128+p]
            nc.sync.dma_start(
                out=lnw_t[:], in_=lnw_d[:].rearrange("(h p) -> p h", p=128)
            )
            rw_t = singles.tile([128, HC, E], F32)  # [p, h, e]
            nc.sync.dma_start(
                out=rw_t[:], in_=rw_d[:, :].rearrange("(h p) e -> p h e", p=128)
            )

            ident = singles.tile([32, 32], F32)
            make_identity(nc, ident[:])

            # ---------------- RMSNorm ----------------
            sq = singles.tile([T, H], F32)
            nc.scalar.activation(out=sq[:], in_=hs_t[:], func=AF.Square)
            ssum = singles.tile([T, 1], F32)
            nc.vector.reduce_sum(out=ssum[:], in_=sq[:], axis=AX.X)
            sd = singles.tile([T, 1], F32)
            eps_t = singles.tile([T, 1], F32)
            nc.vector.memset(eps_t[:], EPS)
            # sqrt(mean(x^2) + eps)
            nc.scalar.activation(
                out=sd[:], in_=ssum[:], func=AF.Sqrt, bias=eps_t[:], scale=1.0 / H
            )
            rstd = singles.tile([T, 1], F32)
            nc.vector.reciprocal(out=rstd[:], in_=sd[:])
            xn = singles.tile([T, H], F32)
            nc.scalar.activation(out=xn[:], in_=hs_t[:], func=AF.Copy, scale=rstd[:])

            # xT[p, (h t)] = x[t, h*128+p] * lnw  (transpose + ln scale)
            xT = singles.tile([128, HC * T], F32)
            xTr = singles.tile([128, HC * T], WDT)
            for h in range(HC):
                pst = ps_tr.tile([128, T], F32)
                nc.tensor.transpose(
                    out=pst[:], in_=xn[:, h * 128:(h + 1) * 128], identity=ident[:]
                )
                nc.scalar.activation(
                    out=xT[:, h * T:(h + 1) * T],
                    in_=pst[:],
                    func=AF.Copy,
                    scale=lnw_t[:, h:h + 1],
                )
                nc.scalar.activation(
                    out=xTr[:, h * T:(h + 1) * T],
                    in_=pst[:],
                    func=AF.Copy,
                    scale=lnw_t[:, h:h + 1],
                )

            def xT_h(h):
                return xT[:, h * T:(h + 1) * T]

            def xTr_h(h):
                return xTr[:, h * T:(h + 1) * T]

            # ---------------- router + top-k affinities ----------------
            ps_r = ps_sm.tile([T, E], F32)
            for h in range(HC):
                # keep the router in plain fp32: selection is discrete
                nc.tensor.matmul(
                    out=ps_r[:],
                    lhsT=xT_h(h),
                    rhs=rw_t[:, h, :],
                    start=(h == 0),
                    stop=(h == HC - 1),
                )
            neg_mx = singles.tile([T, 1], F32)
            nc.vector.tensor_reduce(
                out=neg_mx[:], in_=ps_r[:], axis=AX.X, op=ALU.max, negate=True
            )
            probs = singles.tile([T, E], F32)
            nc.scalar.activation(
                out=probs[:], in_=ps_r[:], func=AF.Exp, bias=neg_mx[:], scale=1.0
            )
            psum_sb = singles.tile([T, 1], F32)
            nc.vector.reduce_sum(out=psum_sb[:], in_=probs[:], axis=AX.X)
            rsum = singles.tile([T, 1], F32)
            nc.vector.reciprocal(out=rsum[:], in_=psum_sb[:])
            nc.vector.tensor_scalar_mul(out=probs[:], in0=probs[:], scalar1=rsum[:])

            work = singles.tile([T, E], F32)
            nc.vector.tensor_copy(out=work[:], in_=probs[:])
            negones = singles.tile([T, E], F32)
            nc.vector.memset(negones[:], -1.0)
            maskt = singles.tile([T, E], mybir.dt.int32)
            mx_i = singles.tile([T, 1], F32)
            for _ in range(TOPK - 1):
                nc.vector.reduce_max(out=mx_i[:], in_=work[:], axis=AX.X)
                nc.vector.tensor_scalar(
                    out=maskt[:], in0=work[:], scalar1=mx_i[:], scalar2=None,
                    op0=ALU.is_ge,
                )
                nc.vector.copy_predicated(out=work[:], mask=maskt[:], data=negones[:])
            t4 = singles.tile([T, 1], F32)
            nc.vector.reduce_max(out=t4[:], in_=work[:], axis=AX.X)
            nc.vector.tensor_scalar(
                out=maskt[:], in0=probs[:], scalar1=t4[:], scalar2=None, op0=ALU.is_ge
            )
            aff = singles.tile([T, E], F32)
            nc.vector.memset(aff[:], 0.0)
            nc.vector.copy_predicated(out=aff[:], mask=maskt[:], data=probs[:])

            # ---------------- experts (2 local) ----------------
            acc = singles.tile([T, H], F32)
            silu_g = singles.tile([T, I], F32)
            h_sb = singles.tile([T, I], F32)
            hT = singles.tile([128, IC * T], WDT)
            tmp_d = singles.tile([T, 512], F32)

            def hT_i(i):
                return hT[:, i * T:(i + 1) * T]

            for e in range(EL):
                aff_col = aff[:, e:e + 1]
                # ---- gate (cols 0..I) then up (cols I..2I) in 2KiB-col passes
                for phase in range(4):           # 0,1 = gate halves; 2,3 = up halves
                    base = phase * 2048
                    is_gate = phase < 2
                    pbank = [ps_mm.tile([T, 512], F32, name=f"pb{n}", tag="pbank") for n in range(4)]
                    for h in range(HC):
                        slab = wslab.tile([128, 2048], WDT)
                        nc.sync.dma_start(
                            out=slab[:],
                            in_=_w(gup_d[e, h * 128:(h + 1) * 128, base:base + 2048]),
                        )
                        for n in range(4):
                            nc.tensor.matmul(
                                out=pbank[n][:],
                                lhsT=xTr_h(h),
                                rhs=slab[:, n * 512:(n + 1) * 512],
                                start=(h == 0),
                                stop=(h == HC - 1),
                            )
                    icol = base if is_gate else base - I
                    for n in range(4):
                        o = icol + n * 512
                        if is_gate:
                            nc.scalar.activation(
                                out=silu_g[:, o:o + 512], in_=pbank[n][:], func=AF.Silu
                            )
                        else:
                            nc.vector.tensor_mul(
                                out=h_sb[:, o:o + 512],
                                in0=silu_g[:, o:o + 512],
                                in1=pbank[n][:],
                            )
                # ---- transpose h to hT
                for i in range(IC):
                    pst = ps_tr.tile([128, T], F32)
                    nc.tensor.transpose(
                        out=pst[:], in_=h_sb[:, i * 128:(i + 1) * 128],
                        identity=ident[:],
                    )
                    nc.scalar.copy(out=hT_i(i), in_=pst[:])
                # ---- down proj, combine with affinity
                pbank = [ps_mm.tile([T, 512], F32, name=f"pb{n}", tag="pbank") for n in range(4)]
                for i in range(IC):
                    slab = wslab.tile([128, 2048], WDT)
                    nc.sync.dma_start(
                        out=slab[:], in_=_w(dwn_d[e, i * 128:(i + 1) * 128, :])
                    )
                    for n in range(4):
                        nc.tensor.matmul(
                            out=pbank[n][:],
                            lhsT=hT_i(i),
                            rhs=slab[:, n * 512:(n + 1) * 512],
                            start=(i == 0),
                            stop=(i == IC - 1),
                        )
                for n in range(4):
                    o = n * 512
                    if e == 0:
                        nc.vector.tensor_scalar(
                            out=acc[:, o:o + 512], in0=pbank[n][:],
                            scalar1=aff_col, scalar2=None, op0=ALU.mult,
                        )
                    else:
                        nc.vector.tensor_scalar(
                            out=tmp_d[:], in0=pbank[n][:],
                            scalar1=aff_col, scalar2=None, op0=ALU.mult,
                        )
                        nc.vector.tensor_add(
                            out=acc[:, o:o + 512], in0=acc[:, o:o + 512],
                            in1=tmp_d[:],
                        )

            # ---------------- shared expert (TP shard of IS) ----------------
            sg_t = sbig.tile([128, HC, ISL], WDT, tag="sbig")
            nc.sync.dma_start(
                out=sg_t[:], in_=_w(sgw_d[:, :].rearrange("(h p) c -> p h c", p=128))
            )
            ps_sg = ps_mm.tile([T, ISL], F32, name="ps_sg", tag="pbank")
            for h in range(HC):
                nc.tensor.matmul(
                    out=ps_sg[:],
                    lhsT=xTr_h(h),
                    rhs=sg_t[:, h, :],
                    start=(h == 0),
                    stop=(h == HC - 1),
                )
            sh_silu = singles.tile([T, ISL], F32)
            nc.scalar.activation(out=sh_silu[:], in_=ps_sg[:], func=AF.Silu)

            su_t = sbig.tile([128, HC, ISL], WDT, tag="sbig")
            nc.sync.dma_start(
                out=su_t[:], in_=_w(suw_d[:, :].rearrange("(h p) c -> p h c", p=128))
            )
            ps_su = ps_mm.tile([T, ISL], F32, name="ps_su", tag="pbank")
            for h in range(HC):
                nc.tensor.matmul(
                    out=ps_su[:],
                    lhsT=xTr_h(h),
                    rhs=su_t[:, h, :],
                    start=(h == 0),
                    stop=(h == HC - 1),
                )
            sh_h = singles.tile([T, ISL], F32)
            nc.vector.tensor_mul(out=sh_h[:], in0=sh_silu[:], in1=ps_su[:])

            shT = singles.tile([128, (ISL // 128) * T], WDT)
            for i in range(ISL // 128):
                pst = ps_tr.tile([128, T], F32)
                nc.tensor.transpose(
                    out=pst[:], in_=sh_h[:, i * 128:(i + 1) * 128], identity=ident[:]
                )
                nc.scalar.copy(out=shT[:, i * T:(i + 1) * T], in_=pst[:])

            sd_t = sbig.tile([128, ISL // 128, H], WDT, tag="sbig")
            nc.sync.dma_start(
                out=sd_t[:], in_=_w(sdw_d[:, :].rearrange("(i p) c -> p i c", p=128))
            )
            pbank = [ps_mm.tile([T, 512], F32, name=f"pb{n}", tag="pbank") for n in range(4)]
            for i in range(ISL // 128):
                for n in range(4):
                    nc.tensor.matmul(
                        out=pbank[n][:],
                        lhsT=shT[:, i * T:(i + 1) * T],
                        rhs=sd_t[:, i, n * 512:(n + 1) * 512],
                        start=(i == 0),
                        stop=(i == ISL // 128 - 1),
                    )
            for n in range(4):
                o = n * 512
                nc.vector.tensor_add(
                    out=acc[:, o:o + 512], in0=acc[:, o:o + 512], in1=pbank[n][:]
                )

            # ---------------- combine across cores ----------------
            nc.sync.dma_start(out=cc_in[:, :], in_=acc[:])
            nc.gpsimd.collective_compute(
                "AllReduce",
                ALU.add,
                replica_groups=[list(range(NCORES))],
                ins=[cc_in[:, :]],
                outs=[cc_out[:, :]],
            )
            nc.sync.dma_start(out=out_d[:, :], in_=cc_out[:, :])

    nc.finalize()
    return nc


_NC_CACHE = {}


def _get_nc():
    if "nc" not in _NC_CACHE:
        _NC_CACHE["nc"] = build_nc()
    return _NC_CACHE["nc"]


def make_in_maps(inputs):
    hs = np.ascontiguousarray(
        np.asarray(inputs["hidden_states"], np.float32).reshape(T, H)
    )
    lnw = np.ascontiguousarray(np.asarray(inputs["ln_weight"], np.float32))
    rw = np.asarray(inputs["router_weight"], np.float32)
    gup = np.asarray(inputs["gate_up_proj"], np.float32)
    dwn = np.asarray(inputs["down_proj"], np.float32)
    sgw = np.asarray(inputs["shared_gate_w"], np.float32)
    suw = np.asarray(inputs["shared_up_w"], np.float32)
    sdw = np.asarray(inputs["shared_down_w"], np.float32)

    in_maps = []
    for r in range(NCORES):
        local = list(range(r * EL, (r + 1) * EL))
        perm = local + [g for g in range(E) if g not in local]
        in_maps.append(
            {
                "hs": hs,
                "lnw": lnw,
                "rw": np.ascontiguousarray(rw[:, perm]),
                "gup": np.ascontiguousarray(gup[local]),
                "dwn": np.ascontiguousarray(dwn[local]),
                "sgw": np.ascontiguousarray(sgw[:, r * ISL:(r + 1) * ISL]),
                "suw": np.ascontiguousarray(suw[:, r * ISL:(r + 1) * ISL]),
                "sdw": np.ascontiguousarray(sdw[r * ISL:(r + 1) * ISL, :]),
            }
        )
    return in_maps


def kernel(**inputs) -> np.ndarray:
    nc = _get_nc()
    in_maps = make_in_maps(inputs)
    res = run_bass_kernel_spmd(nc, in_maps, core_ids=list(range(NCORES)))
    out = res.results[0]["out"]
    return np.asarray(out, np.float32).reshape(B, S, H)


if __name__ == "__main__":
    rng = np.random.default_rng(0)
    ins = {
        "hidden_states": rng.standard_normal((B, S, H), np.float32),
        "ln_weight": np.ones(H, np.float32),
        "router_weight": rng.standard_normal((H, E), np.float32) * 0.02,
        "gate_up_proj": rng.standard_normal((E, H, 2 * I), np.float32) * 0.02,
        "down_proj": rng.standard_normal((E, I, H), np.float32) * 0.02,
        "shared_gate_w": rng.standard_normal((H, IS), np.float32) * 0.02,
        "shared_up_w": rng.standard_normal((H, IS), np.float32) * 0.02,
        "shared_down_w": rng.standard_normal((IS, H), np.float32) * 0.02,
    }
    print(kernel(**ins).shape)
